# revision 49
# baseline (speedup 1.0000x reference)
"""Causal self-attention Trainium2 kernel (B=1, S=4096, E=1024, H=16, D=64).

Sharding: tensor-parallel over heads — 2 heads per core (8 cores).
Each core computes Q/K/V for its 2 heads, causal attention, and a partial
o_proj over its 128 output-feature slice; the host sums the 8 partials.

Device-side structure (per core):
  * x arrives pre-transposed as xT [E, S] bf16 (host does the transpose),
    so every matmul contracts over the partition axis with contiguous DMAs.
  * Q/K kept transposed in SBUF (qts/kts: [128(d of 2 heads), 512] tiles);
    V in normal layout with an appended ones column so the PV matmul also
    accumulates the softmax denominator in PSUM row 64.
  * Logits computed transposed, lg[kv, q] = K @ Q.T, both heads packed
    into PE row-groups (tile_position rows 0/64) writing separate banks.
  * exp is SPLIT between two engines so neither is the bottleneck:
      - ScalarE (ACT): true exp, PSUM->SBUF fp8e5, scale + a -3.5
        range-shift bias folded in (no max-subtraction needed since
        logits ~ N(0,1); the uniform e^bias cancels in normalization).
      - VectorE (DVE): Schraudolph bit-trick — one tensor_scalar
        (code = A*logit + B, f32 -> int8) whose int8 result IS the
        fp8e5m2 encoding of ~exp(scale*logit + bias).  Codes always land
        in [5, 75] so no clamping is needed.  Block assignment per tile
        is tuned so ACT/DVE finish together.
  * Causal masking via GPSIMD affine_select on the diagonal-band blocks;
    fully-masked columns are skipped in logits/exp/PV.
  * PV uses fp8e5 DoubleRow matmuls (contraction 256 = one kv-block PAIR
    per matmul).  Q-tile 0 keeps a bf16 PV path: its low-context rows
    don't average away fp8 noise.
  * PV is emitted two pairs behind its exp, so the in-order PE queue never
    stalls on exp (which would delay logits fills and bubble the engines).
  * Both heads' PV accumulators live in ONE [65,1024] PSUM tile so the
    normalize batches both heads: two denominator-row copies to SBUF (the
    reciprocal custom-op reads garbage from PSUM on HW; instructions must
    not cross PSUM banks), one reciprocal_approx_fast, one stream_shuffle
    broadcast, two multiplies.
  * PSUM->SBUF casts (K/Q/V projections, o_proj outputs) are routed per
    tile phase: VectorE early (before its exp share ramps), ScalarE late.
  * x arrives in deadline-scheduled DMA waves (first 512 cols prioritized
    behind wk so tile-0 compute starts ~10us in); output DMAs ride the
    cheap GPSIMD queue, fanning over three queues for the final drain.
  * o_proj matmuls are BANKED: early tiles contribute units, late tiles
    (whose exp volume leaves the PE short of filler) consume them, and 8
    units interleave into the tail chunks — the PE must never idle ~3.4us
    or the HAM clock gate halves its clock for multi-us stretches.
  * The last tile's tail is pipelined in 128-col chunks, with the first
    half overlapping the final diagonal pair's exp/PV (that pair only
    writes pv columns 256:512).

Measured on 8 axon TRN2 cores: ~211 us HW exec (baseline: 219 us), rel
err ~7.5e-3 vs the fp32 reference (bf16 inputs + fp8e5 attention path).
"""

import math
import sys
from collections import deque

import numpy as np

for _p in ("/opt/trn_rl_repo", "/opt/trn_rl_repo/concourse"):
    if _p not in sys.path:
        sys.path.insert(0, _p)

import ml_dtypes

BF16 = ml_dtypes.bfloat16

S = 4096
E = 1024
H = 16
D = 64
NCORES = 8
DH = 128  # head dims per core (2 heads x 64)
QT = 512  # query tile (free dim of logits matmuls)
NQ = S // QT  # 8
KB = 128  # kv block (partition dim of logits tiles)
SCALE = 1.0 / math.sqrt(D)

# exp(scale*logit + EXP_BIAS): range shift; the uniform e^bias factor
# cancels in the normalization.
EXP_BIAS = -3.5
# Schraudolph constants: int8 code = SCH_A*logit_pre + SCH_B is the
# fp8e5m2 bit pattern of ~exp(SCALE*logit_pre + EXP_BIAS).  e5m2 has a
# 2-bit mantissa (4 codes/octave) and exponent bias 15.  SCH_SIG holds
# the piecewise-linear optimal shift; rounding-mode slack is tuned on HW.
_L2E = 1.4426950408889634
SCH_SIG = -0.17
SCH_A = 4.0 * _L2E * SCALE
SCH_B = 4.0 * (15.0 + _L2E * EXP_BIAS) + SCH_SIG

# blocks per tile whose exp runs on the DVE (of n_kb = 4*(qi+1))
DVE_EXP_CNT = {0: 0, 1: 0, 2: 2, 3: 4, 4: 7, 5: 10, 6: 12, 7: 13}

_CACHE = {}


def _build_nc():
    import concourse.tile as tile
    from concourse import bacc, mybir

    dt = mybir.dt
    f32 = dt.float32
    bf16 = dt.bfloat16
    fp8 = dt.float8e5
    i8 = dt.int8
    Exp = mybir.ActivationFunctionType.Exp
    DoubleRow = mybir.MatmulPerfMode.DoubleRow
    Mult = mybir.AluOpType.mult
    Add = mybir.AluOpType.add

    nc = bacc.Bacc("TRN2", target_bir_lowering=False, debug=False, num_devices=NCORES)

    xT_d = nc.dram_tensor("xT", [E, S], bf16, kind="ExternalInput")
    wq_d = nc.dram_tensor("wq", [128, 1024], bf16, kind="ExternalInput")
    wk_d = nc.dram_tensor("wk", [128, 1024], bf16, kind="ExternalInput")
    wv_d = nc.dram_tensor("wv", [128, 1024], bf16, kind="ExternalInput")
    wo_d = nc.dram_tensor("wo", [128, 1024], bf16, kind="ExternalInput")
    out_d = nc.dram_tensor("out", [S, E], bf16, kind="ExternalOutput")

    with tile.TileContext(nc) as tc:
        from contextlib import ExitStack

        with ExitStack() as ctx:
            sb = ctx.enter_context(tc.tile_pool(name="sb", bufs=1))
            lgp = ctx.enter_context(tc.tile_pool(name="lgp", bufs=2, space="PSUM"))
            ps = ctx.enter_context(tc.tile_pool(name="ps", bufs=2, space="PSUM"))
            pvp = ctx.enter_context(tc.tile_pool(name="pvp", bufs=1, space="PSUM"))
            expp = ctx.enter_context(tc.tile_pool(name="expp", bufs=6))
            normp = ctx.enter_context(tc.tile_pool(name="normp", bufs=3))
            ostp = ctx.enter_context(tc.tile_pool(name="ostp", bufs=4))

            # ---- persistent SBUF tensors + input DMA ----
            wq_sb = sb.tile([128, 1024], bf16, name="wq_sb", tag="wq_sb")
            wk_sb = sb.tile([128, 1024], bf16, name="wk_sb", tag="wk_sb")
            wv_sb = sb.tile([128, 1024], bf16, name="wv_sb", tag="wv_sb")
            wo_sb = sb.tile([128, 1024], bf16, name="wo_sb", tag="wo_sb")
            xts = [
                sb.tile([128, S], bf16, name=f"xt{ec}", tag=f"xt{ec}")
                for ec in range(8)
            ]
            # DMA priority order: wk, then the x columns tile-0's projections
            # need ([0:512] per chunk), then the other weights, then the rest
            # of x.  Alternate DMA queues so transfers overlap.
            nc.sync.dma_start(wk_sb[:], wk_d[:])
            for ec in range(8):
                eng = nc.sync if ec % 2 == 0 else nc.gpsimd
                eng.dma_start(
                    xts[ec][:, 0:QT], xT_d[ec * 128 : (ec + 1) * 128, 0:QT]
                )
            nc.gpsimd.dma_start(wq_sb[:], wq_d[:])
            nc.sync.dma_start(wv_sb[:], wv_d[:])
            nc.gpsimd.dma_start(wo_sb[:], wo_d[:])
            for ec in range(8):
                eng = nc.sync if ec % 2 == 0 else nc.gpsimd
                eng.dma_start(
                    xts[ec][:, QT : 2 * QT],
                    xT_d[ec * 128 : (ec + 1) * 128, QT : 2 * QT],
                )

            # remaining x columns arrive in deadline-scheduled 1024-col
            # waves, emitted at tile starts — spreads DMA bandwidth (power)
            # and keeps each wave comfortably ahead of the projections that
            # consume it.
            def x_wave(w):
                c0 = w * 2 * QT
                for ec in range(8):
                    eng = nc.sync if ec % 2 == 0 else nc.gpsimd
                    eng.dma_start(
                        xts[ec][:, c0 : c0 + 2 * QT],
                        xT_d[ec * 128 : (ec + 1) * 128, c0 : c0 + 2 * QT],
                    )

            kts = [sb.tile([128, QT], bf16, name=f"kt{i}", tag=f"kt{i}") for i in range(NQ)]
            qts = [sb.tile([128, QT], bf16, name=f"qt{i}", tag=f"qt{i}") for i in range(NQ)]
            aots = [sb.tile([128, QT], bf16, name=f"ao{i}", tag=f"ao{i}") for i in range(NQ)]
            # V for DoubleRow PV: one fp8 tile per kv-block PAIR, layout
            # [128(s within block), pair-slot(2) x 160]: head A V at d 0-63 +
            # ones col 64; head B V at 80-143 + ones col 144 (pair-slot
            # stride 160 B keeps the DoubleRow 16B-alignment rule).
            v8s = []
            for i in range(16):
                v = sb.tile([128, 320], fp8, name=f"v{i}", tag=f"v{i}")
                vv = v[:].rearrange("p (t d) -> p t d", t=2)
                nc.vector.memset(vv[:, :, 64:65], 1.0)
                nc.vector.memset(vv[:, :, 144:145], 1.0)
                v8s.append(v)
            # bf16 V for q-tile 0 (its rows have little context, so fp8
            # attention noise doesn't average out there -> keep bf16)
            vb16 = []
            for i in range(4):
                v = sb.tile([128, 130], bf16, name=f"vb{i}", tag=f"vb{i}")
                nc.vector.memset(v[:, 64:65], 1.0)
                nc.vector.memset(v[:, 129:130], 1.0)
                vb16.append(v)

            # per-partition bias column for the exp range shift
            ebias = sb.tile([128, 1], f32, name="ebias", tag="ebias")
            nc.vector.memset(ebias[:], EXP_BIAS)

            # ---- filler-unit constructors (projections / o_proj) ----
            def kq_units(dst, w, st, cast_eng="v"):
                cols = slice(st * QT, (st + 1) * QT)
                state = {}

                def mm(ec):
                    def f():
                        if ec == 0:
                            state["t"] = ps.tile([128, QT], f32, name="ps_kq", tag="ps")
                        nc.tensor.matmul(
                            state["t"][:],
                            lhsT=w[:, ec * 128 : (ec + 1) * 128],
                            rhs=xts[ec][:, cols],
                            start=(ec == 0),
                            stop=(ec == 7),
                        )

                    return f

                def cast():
                    if cast_eng == "s":
                        nc.scalar.copy(dst[:], state["t"][:])
                    else:
                        nc.vector.tensor_copy(dst[:], state["t"][:])

                return [mm(ec) for ec in range(8)] + [cast]

            def v_units(kb, cast_eng="v"):
                state = {}

                def mm(ec):
                    def f():
                        if ec == 0:
                            state["t"] = ps.tile([128, 128], f32, name="ps_v", tag="ps")
                        nc.tensor.matmul(
                            state["t"][:],
                            lhsT=xts[ec][:, kb * 128 : (kb + 1) * 128],
                            rhs=wv_sb[:, ec * 128 : (ec + 1) * 128],
                            start=(ec == 0),
                            stop=(ec == 7),
                        )

                    return f

                def cast():
                    # both pair-slot segments in one strided copy:
                    # [128, 2(head), 64] with head stride 80 inside the slot
                    vv = v8s[kb // 2][:].rearrange("p (t h e) -> p t h e", t=2, h=2)
                    src = state["t"][:].rearrange("p (h e) -> p h e", h=2)
                    r = kb % 2
                    eng = nc.scalar if cast_eng == "s" else nc.vector
                    if cast_eng == "s":
                        eng.copy(vv[:, r, :, 0:64], src[:])
                    else:
                        eng.tensor_copy(vv[:, r, :, 0:64], src[:])
                    if kb < 4:
                        vb = vb16[kb][:].rearrange("p (h e) -> p h e", h=2, e=65)
                        if cast_eng == "s":
                            eng.copy(vb[:, :, 0:64], src[:])
                        else:
                            eng.tensor_copy(vb[:, :, 0:64], src[:])

                return [mm(ec) for ec in range(8)] + [cast]

            def oproj_unit(qj, sbi, half, cast_eng, dma_eng):
                def f():
                    srow = qj * QT + sbi * 128
                    po = ps.tile([128, 512], f32, name="po", tag="ps")
                    nc.tensor.matmul(
                        po[:],
                        lhsT=aots[qj][:, sbi * 128 : (sbi + 1) * 128],
                        rhs=wo_sb[:, half * 512 : (half + 1) * 512],
                        start=True,
                        stop=True,
                    )
                    ost = ostp.tile([128, 512], bf16, name="ost", tag="ost")
                    if cast_eng == "s":
                        nc.scalar.copy(ost[:], po[:])
                    else:
                        nc.vector.tensor_copy(ost[:], po[:])
                    dma_eng.dma_start(
                        out_d[srow : srow + 128, half * 512 : (half + 1) * 512],
                        ost[:],
                    )

                return f

            def oproj_units(qj, sbis=range(4), cast_eng="v", dma_engs=None):
                units = []
                for ui, (sbi, half) in enumerate(
                    (s, h) for s in sbis for h in range(2)
                ):
                    ce = "s" if cast_eng == "s" or (cast_eng == "a" and half == 0) else "v"
                    de = (
                        dma_engs[ui % len(dma_engs)]
                        if dma_engs
                        else (nc.gpsimd if ui % 2 else nc.sync)
                    )
                    units.append(oproj_unit(qj, sbi, half, ce, de))
                return units

            def proj_units(qi2):
                # projections for tile qi2, emitted during tile qi2-1:
                # DVE has slack in the early phase (before its exp share
                # ramps up), ScalarE in the late phase (exp offloaded).
                ce = "v" if qi2 <= 4 else "s"
                u = []
                u += kq_units(kts[qi2], wk_sb, qi2, cast_eng=ce)
                u += kq_units(qts[qi2], wq_sb, qi2, cast_eng=ce)
                for kb in range(4 * qi2, 4 * qi2 + 4):
                    u += v_units(kb, cast_eng="v" if kb < 20 else "s")
                return u

            # seed tile for the denominator-reciprocal broadcast
            bcseed = sb.tile([64, 2 * QT], f32, name="bcseed", tag="bcseed")
            nc.vector.memset(bcseed[:], 0.0)

            def norm_emit(pv2, qj, c0, c1):
                # aot[:, c0:c1] = pv[0:64] / pv[64] for both heads at once
                # (head A in pv2 cols 0:512, head B in 512:1024): per-bank
                # denominator copies to SBUF (the reciprocal custom-op reads
                # garbage from PSUM on HW, and single instructions must not
                # cross PSUM banks), then one batched reciprocal, one
                # quadrant broadcast (stream_shuffle), two multiplies.
                den_sb = normp.tile([1, 2 * QT], f32, name="den_sb", tag="den")
                if qj >= 5:
                    nc.scalar.copy(den_sb[:, c0:c1], pv2[64:65, c0:c1])
                    nc.scalar.copy(
                        den_sb[:, QT + c0 : QT + c1], pv2[64:65, QT + c0 : QT + c1]
                    )
                else:
                    nc.vector.tensor_copy(den_sb[:, c0:c1], pv2[64:65, c0:c1])
                    nc.vector.tensor_copy(
                        den_sb[:, QT + c0 : QT + c1], pv2[64:65, QT + c0 : QT + c1]
                    )
                den_v = den_sb[:].rearrange("p (b q) -> p b q", b=2)[:, :, c0:c1]
                seed0 = bcseed[0:1, :].rearrange("p (b q) -> p b q", b=2)[
                    :, :, c0:c1
                ]
                seed32 = bcseed[32:33, :].rearrange("p (b q) -> p b q", b=2)[
                    :, :, c0:c1
                ]
                nc.vector.reciprocal_approx_fast(seed0, den_v)
                nc.vector.tensor_copy(seed32, seed0)
                bcast = normp.tile([64, 2 * QT], f32, name="bcast", tag="bcast")
                bc_v = bcast[:].rearrange("p (b q) -> p b q", b=2)[:, :, c0:c1]
                sd_v = bcseed[:].rearrange("p (b q) -> p b q", b=2)[:, :, c0:c1]
                nc.vector.stream_shuffle(bc_v, sd_v, [0] * 32)
                nc.vector.tensor_mul(
                    aots[qj][0:64, c0:c1], pv2[0:64, c0:c1], bcast[:, c0:c1]
                )
                nc.vector.tensor_mul(
                    aots[qj][64:128, c0:c1], pv2[0:64, QT + c0 : QT + c1],
                    bcast[:, QT + c0 : QT + c1],
                )

            def dve_exp_blocks(qi, n_kb):
                cnt = DVE_EXP_CNT.get(qi, 0)
                return {
                    kb
                    for kb in range(n_kb)
                    if ((kb + 1) * cnt) // n_kb > (kb * cnt) // n_kb
                }

            # ---- prologue: K/Q projections for q-tile 0 (dense PE warmup);
            # V blocks 0-3 are fillers inside the tile-0 loop.
            for f in kq_units(kts[0], wk_sb, 0, cast_eng="v"):
                f()
            for f in kq_units(qts[0], wq_sb, 0, cast_eng="v"):
                f()

            # ---- main loop over q-tiles ----
            # o_proj work is BANKED: each tile contributes 8 units when its
            # aot is ready, but late tiles consume more — they are the ones
            # whose exp volume leaves the PE short of filler work.
            oproj_backlog = deque()
            # tile 7 keeps only 4 as block fillers — the other 8 interleave
            # into the tail chunks to keep the PE continuously busy (HAM!)
            OPROJ_CONSUME = {1: 4, 2: 4, 3: 4, 4: 8, 5: 12, 6: 12, 7: 4}
            for qi in range(NQ):
                if qi <= 2:
                    x_wave(qi + 1)
                fillers = deque()
                if qi == 0:
                    for kb in range(4):
                        fillers.extend(v_units(kb, cast_eng="v"))
                if qi + 1 < NQ:
                    fillers.extend(proj_units(qi + 1))
                if qi >= 1:
                    for sbi in range(4):
                        for half in range(2):
                            oproj_backlog.append((qi - 1, sbi, half))
                    n_cons = min(OPROJ_CONSUME[qi], len(oproj_backlog))
                    for ci in range(n_cons):
                        qj, sbi, half = oproj_backlog.popleft()
                        ce = "v" if qi <= 4 else ("s" if ci % 2 == 0 else "v")
                        de = nc.gpsimd if ci % 2 else nc.sync
                        fillers.append(oproj_unit(qj, sbi, half, ce, de))

                n_kb = 4 * (qi + 1)
                # both heads' PV accumulators in ONE psum tile (2 banks):
                # head A cols 0:512, head B cols 512:1024 — lets the
                # normalize batch both heads per instruction
                pv2 = pvp.tile([65, 2 * QT], f32, name="pv2", tag="pv")
                if qi == 0:
                    # bf16 path for the first q-tile (rows 0-511): every
                    # block is diagonal; per-block M=65 PV, no DoubleRow.
                    for kb in range(4):
                        kvs = slice(kb * KB, (kb + 1) * KB)
                        off = kb
                        qlo = off * KB
                        nq = QT - qlo
                        lg = lgp.tile([128, 2 * QT], f32, name="lg", tag="lg")
                        nc.tensor.matmul(
                            lg[:, qlo:QT], lhsT=kts[0][0:64, kvs],
                            rhs=qts[0][0:64, qlo:QT], start=True, stop=True,
                        )
                        nc.tensor.matmul(
                            lg[:, QT + qlo : 2 * QT], lhsT=kts[0][64:128, kvs],
                            rhs=qts[0][64:128, qlo:QT], start=True, stop=True,
                        )
                        exb = expp.tile([128, 2 * QT], bf16, name="exb", tag="exp")
                        lg_v = lg[:].rearrange("p (h q) -> p h q", h=2)[:, :, qlo:QT]
                        exb_v = exb[:].rearrange("p (h q) -> p h q", h=2)[:, :, qlo:QT]
                        nc.scalar.activation(exb_v, lg_v, Exp, scale=SCALE,
                                             bias=ebias[:])
                        if fillers:
                            n_pop = math.ceil(len(fillers) / (4 - kb))
                            for _ in range(n_pop):
                                fillers.popleft()()
                        # only the 128 columns crossing the diagonal need
                        # masking; later columns are fully kept as-is
                        exb_w = exb[:].rearrange("p (h q) -> p h q", h=2)[
                            :, :, qlo : qlo + KB
                        ]
                        nc.gpsimd.affine_select(
                            out=exb_w, in_=exb_w,
                            compare_op=mybir.AluOpType.is_ge,
                            fill=0.0, base=0,
                            pattern=[[0, 2], [1, KB]],
                            channel_multiplier=-1,
                        )
                        nc.tensor.matmul(
                            pv2[:, qlo:QT], lhsT=vb16[kb][:, 0:65],
                            rhs=exb[:, qlo:QT],
                            start=(kb == 0), stop=(kb == 3),
                            skip_group_check=True,
                        )
                        nc.tensor.matmul(
                            pv2[:, QT + qlo : 2 * QT], lhsT=vb16[kb][:, 65:130],
                            rhs=exb[:, QT + qlo : 2 * QT],
                            start=(kb == 0), stop=(kb == 3),
                            skip_group_check=True,
                        )
                    n_kb = 0  # skip the fp8 loop below

                dve_set = dve_exp_blocks(qi, n_kb)
                ex8 = None
                pending_pvs = deque()
                for kb in range(n_kb):
                    # logits for both heads of kv-block kb: head A -> cols
                    # 0:512 (PSUM bank 0), head B -> cols 512:1024 (bank 1).
                    # Row-group packing (rows 0-63 / 64-127) runs the two
                    # matmuls concurrently in the PE array.
                    # Diagonal-band pairs: columns q < qlo_p = (pair off)*128
                    # are entirely masked for both members -> skip them.
                    kvs = slice((kb % 4) * KB, (kb % 4 + 1) * KB)
                    ktile = kts[kb // 4]
                    r = kb % 2
                    off = kb - 4 * qi
                    qlo = max(off - r, 0) * KB  # pair-aligned trim
                    nq = QT - qlo
                    lg = lgp.tile([128, 2 * QT], f32, name="lg", tag="lg")
                    nc.tensor.matmul(
                        lg[:, qlo:QT], lhsT=ktile[0:64, kvs],
                        rhs=qts[qi][0:64, qlo:QT],
                        start=True, stop=True,
                    )
                    nc.tensor.matmul(
                        lg[:, QT + qlo : 2 * QT], lhsT=ktile[64:128, kvs],
                        rhs=qts[qi][64:128, qlo:QT],
                        start=True, stop=True,
                    )
                    if r == 0:
                        # exp tile for this kv pair: [128, (head, slot, q)]
                        ex8 = expp.tile([128, 4 * QT], fp8, name="ex8", tag="exp")
                    exv = ex8[:].rearrange("p (h t q) -> p h t q", h=2, t=2)
                    lg_v = lg[:].rearrange("p (h q) -> p h q", h=2)[:, :, qlo:QT]
                    ex_v = exv[:, :, r, qlo:QT]
                    if kb in dve_set:
                        # Schraudolph on DVE: int8 code IS the e5m2 encoding
                        nc.vector.tensor_scalar(
                            out=ex_v.bitcast(i8),
                            in0=lg_v,
                            scalar1=SCH_A,
                            scalar2=SCH_B,
                            op0=Mult,
                            op1=Add,
                        )
                    else:
                        nc.scalar.activation(ex_v, lg_v, Exp, scale=SCALE,
                                             bias=ebias[:])

                    # emit PV pairs TWO pairs behind their exp: the PE
                    # reaches them well after both engines' exps finished,
                    # so the in-order PE queue never stalls on exp (which
                    # would delay logits fills and bubble ACT/DVE)
                    if r == 1 and len(pending_pvs) >= 2:
                        pending_pvs.popleft()()

                    # PE filler work while ACT/DVE compute exp
                    if fillers:
                        n_pop = math.ceil(len(fillers) / (n_kb - kb))
                        for _ in range(n_pop):
                            fillers.popleft()()

                    if off >= 0:
                        # causal mask on GPSIMD: keep where q - kv - off*128
                        # >= 0 else 0; with q = qlo + j this is
                        # j + qlo - off*128 - kv >= 0.  Only columns up to
                        # the end of the diagonal band (q < off*128+128) can
                        # be masked — don't touch the fully-kept tail.
                        w = min(off * KB + KB - qlo, nq)
                        ex_w = exv[:, :, r, qlo : qlo + w]
                        nc.gpsimd.affine_select(
                            out=ex_w,
                            in_=ex_w,
                            compare_op=mybir.AluOpType.is_ge,
                            fill=0.0,
                            base=qlo - off * KB,
                            pattern=[[0, 2], [1, w]],
                            channel_multiplier=-1,
                        )
                    if r == 1:
                        # DoubleRow PV over the kv pair (contraction 256):
                        # lhsT [128, 2, 65], rhs [128, 2, nq] -> out [65, nq]
                        def make_pv(kp=kb // 2, qlo=qlo, exv=exv, last=(kb == n_kb - 1)):
                            def f():
                                vv = v8s[kp][:].rearrange("p (t d) -> p t d", t=2)
                                nc.tensor.matmul(
                                    pv2[:, qlo:QT], lhsT=vv[:, :, 0:65],
                                    rhs=exv[:, 0, :, qlo:QT],
                                    start=(kp == 0), stop=last,
                                    perf_mode=DoubleRow,
                                    skip_group_check=True,
                                )
                                nc.tensor.matmul(
                                    pv2[:, QT + qlo : 2 * QT], lhsT=vv[:, :, 80:145],
                                    rhs=exv[:, 1, :, qlo:QT],
                                    start=(kp == 0), stop=last,
                                    perf_mode=DoubleRow,
                                    skip_group_check=True,
                                )

                            return f

                        pending_pvs.append(make_pv())
                if qi == NQ - 1 and len(pending_pvs) == 2:
                    # flush only the second-to-last pair: it finalizes pv
                    # columns 0:256 (the last diagonal pair writes 256:512
                    # only), so the first half of the tail can overlap it
                    pending_pvs.popleft()()
                else:
                    while pending_pvs:
                        pending_pvs.popleft()()
                while fillers:
                    fillers.popleft()()
                # normalize: aot = pv[0:64] / pv[64]; the first half's o_proj
                # for the final tile starts while the second half normalizes
                if qi == NQ - 1:
                    # pipeline the tail in 128-col chunks: each chunk's
                    # normalize feeds its own o_proj unit immediately, and
                    # the final DMAs fan out over three queues.
                    tail_dma = [nc.gpsimd, nc.sync, nc.scalar]
                    for sbi in range(4):
                        if sbi == 2:
                            # columns 256:512 need the last diagonal pair —
                            # its exp/PV overlapped the first two chunks
                            while pending_pvs:
                                pending_pvs.popleft()()
                        norm_emit(pv2, qi, sbi * 128, (sbi + 1) * 128)
                        for half in range(2):
                            de = tail_dma[(2 * sbi + half) % 3]
                            oproj_unit(qi, sbi, half, "s", de)()
                        # interleave banked o_proj work between tail chunks
                        # so the PE never idles long enough to re-throttle
                        for _ in range(2):
                            if oproj_backlog:
                                qj, sb_, hf = oproj_backlog.popleft()
                                oproj_unit(qj, sb_, hf, "s", tail_dma[sbi % 3])()
                else:
                    norm_emit(pv2, qi, 0, QT)

    nc.compile()
    return nc


def _host_inputs(x, Wq, Wk, Wv, Wo):
    x2 = np.asarray(x, dtype=np.float32).reshape(S, E)
    xT = np.ascontiguousarray(x2.T).astype(BF16)

    in_maps = []
    for c in range(NCORES):
        r = slice(128 * c, 128 * (c + 1))

        def pack(wT):  # [1024(e), 128(d)] -> [128(p), ec*128+d]
            return np.ascontiguousarray(
                wT.reshape(8, 128, 128).transpose(1, 0, 2).reshape(128, 1024)
            ).astype(BF16)

        wq_c = pack(np.asarray(Wq, np.float32)[r, :].T)
        wk_c = pack(np.asarray(Wk, np.float32)[r, :].T)
        wv_c = pack(np.asarray(Wv, np.float32)[r, :].T)
        wo_c = np.ascontiguousarray(np.asarray(Wo, np.float32)[:, r].T).astype(BF16)
        in_maps.append(
            {
                "xT": xT,
                "wq": wq_c,
                "wk": wk_c,
                "wv": wv_c,
                "wo": wo_c,
            }
        )
    return in_maps


def _get_nc():
    if "nc" not in _CACHE:
        _CACHE["nc"] = _build_nc()
    return _CACHE["nc"]


def run(x, Wq, Wk, Wv, Wo, trace=False, trace_kwargs=None):
    """Build+run the SPMD kernel; returns (full_output [S,E] f32, BassKernelResults)."""
    from concourse.bass_utils import run_bass_kernel_spmd

    nc = _get_nc()
    in_maps = _host_inputs(x, Wq, Wk, Wv, Wo)
    res = run_bass_kernel_spmd(
        nc,
        in_maps,
        list(range(NCORES)),
        trace=trace,
        **(trace_kwargs or {}),
    )
    out = np.zeros((S, E), dtype=np.float32)
    for c in range(NCORES):
        out += res.results[c]["out"].astype(np.float32)
    return out, res


def kernel(x, Wq, Wk, Wv, Wo):
    out, _ = run(x, Wq, Wk, Wv, Wo)
    return out.reshape(1, S, E).astype(np.float32)


# revision 53
# speedup vs baseline: 1.0262x; 1.0262x over previous
"""Causal self-attention Trainium2 kernel (B=1, S=4096, E=1024, H=16, D=64).

Sharding: tensor-parallel over heads — 2 heads per core (8 cores).
Each core computes Q/K/V for its 2 heads, causal attention, and a partial
o_proj over its 128 output-feature slice; the host sums the 8 partials.

Device-side structure (per core):
  * x arrives pre-transposed as xT [E, S] bf16 (host does the transpose),
    so every matmul contracts over the partition axis with contiguous DMAs.
  * Q/K kept transposed in SBUF (qts/kts: [128(d of 2 heads), 512] tiles);
    V in normal layout with an appended ones column so the PV matmul also
    accumulates the softmax denominator in PSUM row 64.
  * Logits computed transposed, lg[kv, q] = K @ Q.T, both heads packed
    into PE row-groups (tile_position rows 0/64) writing separate banks.
  * exp is SPLIT between two engines so neither is the bottleneck:
      - ScalarE (ACT): true exp, PSUM->SBUF fp8e5, scale + a -3.5
        range-shift bias folded in (no max-subtraction needed since
        logits ~ N(0,1); the uniform e^bias cancels in normalization).
      - VectorE (DVE): Schraudolph bit-trick — one tensor_scalar
        (code = A*logit + B, f32 -> int8) whose int8 result IS the
        fp8e5m2 encoding of ~exp(scale*logit + bias).  Codes always land
        in [5, 75] so no clamping is needed.  Block assignment per tile
        is tuned so ACT/DVE finish together.
  * Causal masking via GPSIMD affine_select on the diagonal-band blocks;
    fully-masked columns are skipped in logits/exp/PV.
  * PV uses fp8e5 DoubleRow matmuls (contraction 256 = one kv-block PAIR
    per matmul).  Q-tile 0 keeps a bf16 PV path: its low-context rows
    don't average away fp8 noise.
  * PV is emitted two pairs behind its exp, so the in-order PE queue never
    stalls on exp (which would delay logits fills and bubble the engines).
  * Both heads' PV accumulators live in ONE [65,1024] PSUM tile so the
    normalize batches both heads: two denominator-row copies to SBUF (the
    reciprocal custom-op reads garbage from PSUM on HW; instructions must
    not cross PSUM banks), one reciprocal_approx_fast, one stream_shuffle
    broadcast, two multiplies.
  * PSUM->SBUF casts (K/Q/V projections, o_proj outputs) are routed per
    tile phase: VectorE early (before its exp share ramps), ScalarE late.
  * x arrives in deadline-scheduled DMA waves (first 512 cols prioritized
    behind wk so tile-0 compute starts ~10us in); output DMAs ride the
    cheap GPSIMD queue, fanning over three queues for the final drain.
  * o_proj matmuls are BANKED: early tiles contribute units, late tiles
    (whose exp volume leaves the PE short of filler) consume them, and 8
    units interleave into the tail chunks — the PE must never idle ~3.4us
    or the HAM clock gate halves its clock for multi-us stretches.
  * The last tile's tail is pipelined in 128-col chunks, with the first
    half overlapping the final diagonal pair's exp/PV (that pair only
    writes pv columns 256:512).

Measured on 8 axon TRN2 cores: ~211 us HW exec (baseline: 219 us), rel
err ~7.5e-3 vs the fp32 reference (bf16 inputs + fp8e5 attention path).
"""

import math
import sys
from collections import deque

import numpy as np

for _p in ("/opt/trn_rl_repo", "/opt/trn_rl_repo/concourse"):
    if _p not in sys.path:
        sys.path.insert(0, _p)

import ml_dtypes

BF16 = ml_dtypes.bfloat16

S = 4096
E = 1024
H = 16
D = 64
NCORES = 8
DH = 128  # head dims per core (2 heads x 64)
QT = 512  # query tile (free dim of logits matmuls)
NQ = S // QT  # 8
KB = 128  # kv block (partition dim of logits tiles)
SCALE = 1.0 / math.sqrt(D)

# exp(scale*logit + EXP_BIAS): range shift; the uniform e^bias factor
# cancels in the normalization.
EXP_BIAS = -3.5
# Schraudolph constants: int8 code = SCH_A*logit_pre + SCH_B is the
# fp8e5m2 bit pattern of ~exp(SCALE*logit_pre + EXP_BIAS).  e5m2 has a
# 2-bit mantissa (4 codes/octave) and exponent bias 15.  SCH_SIG holds
# the piecewise-linear optimal shift; rounding-mode slack is tuned on HW.
_L2E = 1.4426950408889634
SCH_SIG = -0.17
SCH_A = 4.0 * _L2E * SCALE
SCH_B = 4.0 * (15.0 + _L2E * EXP_BIAS) + SCH_SIG

# blocks per tile whose exp runs on the DVE (of n_kb = 4*(qi+1))
DVE_EXP_CNT = {0: 0, 1: 0, 2: 2, 3: 4, 4: 7, 5: 10, 6: 12, 7: 13}

_CACHE = {}


def _build_nc():
    import concourse.tile as tile
    from concourse import bacc, mybir

    dt = mybir.dt
    f32 = dt.float32
    bf16 = dt.bfloat16
    fp8 = dt.float8e5
    i8 = dt.int8
    Exp = mybir.ActivationFunctionType.Exp
    DoubleRow = mybir.MatmulPerfMode.DoubleRow
    Mult = mybir.AluOpType.mult
    Add = mybir.AluOpType.add

    nc = bacc.Bacc("TRN2", target_bir_lowering=False, debug=False, num_devices=NCORES)

    xT_d = nc.dram_tensor("xT", [E, S], bf16, kind="ExternalInput")
    wq_d = nc.dram_tensor("wq", [128, 1024], bf16, kind="ExternalInput")
    wk_d = nc.dram_tensor("wk", [128, 1024], bf16, kind="ExternalInput")
    wv_d = nc.dram_tensor("wv", [128, 1024], bf16, kind="ExternalInput")
    wo_d = nc.dram_tensor("wo", [128, 1024], bf16, kind="ExternalInput")
    out_d = nc.dram_tensor("out", [S, E], bf16, kind="ExternalOutput")

    with tile.TileContext(nc) as tc:
        from contextlib import ExitStack

        with ExitStack() as ctx:
            sb = ctx.enter_context(tc.tile_pool(name="sb", bufs=1))
            lgp = ctx.enter_context(tc.tile_pool(name="lgp", bufs=2, space="PSUM"))
            ps = ctx.enter_context(tc.tile_pool(name="ps", bufs=2, space="PSUM"))
            pvp = ctx.enter_context(tc.tile_pool(name="pvp", bufs=1, space="PSUM"))
            expp = ctx.enter_context(tc.tile_pool(name="expp", bufs=6))
            normp = ctx.enter_context(tc.tile_pool(name="normp", bufs=3))
            ostp = ctx.enter_context(tc.tile_pool(name="ostp", bufs=4))

            # ---- persistent SBUF tensors + input DMA ----
            wq_sb = sb.tile([128, 1024], bf16, name="wq_sb", tag="wq_sb")
            wk_sb = sb.tile([128, 1024], bf16, name="wk_sb", tag="wk_sb")
            wv_sb = sb.tile([128, 1024], bf16, name="wv_sb", tag="wv_sb")
            wo_sb = sb.tile([128, 1024], bf16, name="wo_sb", tag="wo_sb")
            xts = [
                sb.tile([128, S], bf16, name=f"xt{ec}", tag=f"xt{ec}")
                for ec in range(8)
            ]
            # DMA priority order: wk, then the x columns tile-0's projections
            # need ([0:512] per chunk), then the other weights, then the rest
            # of x.  Alternate DMA queues so transfers overlap.
            nc.sync.dma_start(wk_sb[:], wk_d[:])
            for ec in range(8):
                eng = nc.sync if ec % 2 == 0 else nc.gpsimd
                eng.dma_start(
                    xts[ec][:, 0:QT], xT_d[ec * 128 : (ec + 1) * 128, 0:QT]
                )
            nc.gpsimd.dma_start(wq_sb[:], wq_d[:])
            nc.sync.dma_start(wv_sb[:], wv_d[:])
            nc.gpsimd.dma_start(wo_sb[:], wo_d[:])
            for ec in range(8):
                eng = nc.sync if ec % 2 == 0 else nc.gpsimd
                eng.dma_start(
                    xts[ec][:, QT : 2 * QT],
                    xT_d[ec * 128 : (ec + 1) * 128, QT : 2 * QT],
                )

            # remaining x columns arrive in deadline-scheduled 1024-col
            # waves, emitted at tile starts — spreads DMA bandwidth (power)
            # and keeps each wave comfortably ahead of the projections that
            # consume it.
            def x_wave(w):
                c0 = w * 2 * QT
                for ec in range(8):
                    eng = nc.sync if ec % 2 == 0 else nc.gpsimd
                    eng.dma_start(
                        xts[ec][:, c0 : c0 + 2 * QT],
                        xT_d[ec * 128 : (ec + 1) * 128, c0 : c0 + 2 * QT],
                    )

            kts = [sb.tile([128, QT], bf16, name=f"kt{i}", tag=f"kt{i}") for i in range(NQ)]
            qts = [sb.tile([128, QT], bf16, name=f"qt{i}", tag=f"qt{i}") for i in range(NQ)]
            aots = [sb.tile([128, QT], bf16, name=f"ao{i}", tag=f"ao{i}") for i in range(NQ)]
            # V for DoubleRow PV: one fp8 tile per kv-block PAIR, layout
            # [128(s within block), pair-slot(2) x 160]: head A V at d 0-63 +
            # ones col 64; head B V at 80-143 + ones col 144 (pair-slot
            # stride 160 B keeps the DoubleRow 16B-alignment rule).
            v8s = []
            for i in range(16):
                v = sb.tile([128, 320], fp8, name=f"v{i}", tag=f"v{i}")
                vv = v[:].rearrange("p (t d) -> p t d", t=2)
                nc.vector.memset(vv[:, :, 64:65], 1.0)
                nc.vector.memset(vv[:, :, 144:145], 1.0)
                v8s.append(v)
            # bf16 V for q-tile 0 (its rows have little context, so fp8
            # attention noise doesn't average out there -> keep bf16)
            vb16 = []
            for i in range(4):
                v = sb.tile([128, 130], bf16, name=f"vb{i}", tag=f"vb{i}")
                nc.vector.memset(v[:, 64:65], 1.0)
                nc.vector.memset(v[:, 129:130], 1.0)
                vb16.append(v)

            # per-partition bias column for the exp range shift
            ebias = sb.tile([128, 1], f32, name="ebias", tag="ebias")
            nc.vector.memset(ebias[:], EXP_BIAS)

            # ---- filler-unit constructors (projections / o_proj) ----
            def kq_units(dst, w, st, cast_eng="v"):
                cols = slice(st * QT, (st + 1) * QT)
                state = {}

                def mm(ec):
                    def f():
                        if ec == 0:
                            state["t"] = ps.tile([128, QT], f32, name="ps_kq", tag="ps")
                        nc.tensor.matmul(
                            state["t"][:],
                            lhsT=w[:, ec * 128 : (ec + 1) * 128],
                            rhs=xts[ec][:, cols],
                            start=(ec == 0),
                            stop=(ec == 7),
                        )

                    return f

                def cast():
                    if cast_eng == "s":
                        nc.scalar.copy(dst[:], state["t"][:])
                    else:
                        nc.vector.tensor_copy(dst[:], state["t"][:])

                return [mm(ec) for ec in range(8)] + [cast]

            def v_units(kb, cast_eng="v"):
                state = {}

                def mm(ec):
                    def f():
                        if ec == 0:
                            state["t"] = ps.tile([128, 128], f32, name="ps_v", tag="ps")
                        nc.tensor.matmul(
                            state["t"][:],
                            lhsT=xts[ec][:, kb * 128 : (kb + 1) * 128],
                            rhs=wv_sb[:, ec * 128 : (ec + 1) * 128],
                            start=(ec == 0),
                            stop=(ec == 7),
                        )

                    return f

                def cast():
                    # both pair-slot segments in one strided copy:
                    # [128, 2(head), 64] with head stride 80 inside the slot
                    vv = v8s[kb // 2][:].rearrange("p (t h e) -> p t h e", t=2, h=2)
                    src = state["t"][:].rearrange("p (h e) -> p h e", h=2)
                    r = kb % 2
                    eng = nc.scalar if cast_eng == "s" else nc.vector
                    if cast_eng == "s":
                        eng.copy(vv[:, r, :, 0:64], src[:])
                    else:
                        eng.tensor_copy(vv[:, r, :, 0:64], src[:])
                    if kb < 4:
                        vb = vb16[kb][:].rearrange("p (h e) -> p h e", h=2, e=65)
                        if cast_eng == "s":
                            eng.copy(vb[:, :, 0:64], src[:])
                        else:
                            eng.tensor_copy(vb[:, :, 0:64], src[:])

                return [mm(ec) for ec in range(8)] + [cast]

            def oproj_unit(qj, sbi, half, cast_eng, dma_eng):
                def f():
                    srow = qj * QT + sbi * 128
                    po = ps.tile([128, 512], f32, name="po", tag="ps")
                    nc.tensor.matmul(
                        po[:],
                        lhsT=aots[qj][:, sbi * 128 : (sbi + 1) * 128],
                        rhs=wo_sb[:, half * 512 : (half + 1) * 512],
                        start=True,
                        stop=True,
                    )
                    ost = ostp.tile([128, 512], bf16, name="ost", tag="ost")
                    if cast_eng == "s":
                        nc.scalar.copy(ost[:], po[:])
                    else:
                        nc.vector.tensor_copy(ost[:], po[:])
                    dma_eng.dma_start(
                        out_d[srow : srow + 128, half * 512 : (half + 1) * 512],
                        ost[:],
                    )

                return f

            def oproj_units(qj, sbis=range(4), cast_eng="v", dma_engs=None):
                units = []
                for ui, (sbi, half) in enumerate(
                    (s, h) for s in sbis for h in range(2)
                ):
                    ce = "s" if cast_eng == "s" or (cast_eng == "a" and half == 0) else "v"
                    de = (
                        dma_engs[ui % len(dma_engs)]
                        if dma_engs
                        else (nc.gpsimd if ui % 2 else nc.sync)
                    )
                    units.append(oproj_unit(qj, sbi, half, ce, de))
                return units

            def proj_units(qi2):
                # projections for tile qi2, emitted during tile qi2-1:
                # DVE has slack in the early phase (before its exp share
                # ramps up), ScalarE in the late phase (exp offloaded).
                ce = "v" if qi2 <= 4 else "s"
                u = []
                u += kq_units(kts[qi2], wk_sb, qi2, cast_eng=ce)
                u += kq_units(qts[qi2], wq_sb, qi2, cast_eng=ce)
                for kb in range(4 * qi2, 4 * qi2 + 4):
                    u += v_units(kb, cast_eng="v" if kb < 20 else "s")
                return u

            # seed tile for the denominator-reciprocal broadcast
            bcseed = sb.tile([64, 2 * QT], f32, name="bcseed", tag="bcseed")
            nc.vector.memset(bcseed[:], 0.0)

            def norm_emit(pv2, qj, c0, c1):
                # aot[:, c0:c1] = pv[0:64] / pv[64] for both heads at once
                # (head A in pv2 cols 0:512, head B in 512:1024): per-bank
                # denominator copies to SBUF (the reciprocal custom-op reads
                # garbage from PSUM on HW, and single instructions must not
                # cross PSUM banks), then one batched reciprocal, one
                # quadrant broadcast (stream_shuffle), two multiplies.
                den_sb = normp.tile([1, 2 * QT], f32, name="den_sb", tag="den")
                if qj >= 5:
                    nc.scalar.copy(den_sb[:, c0:c1], pv2[64:65, c0:c1])
                    nc.scalar.copy(
                        den_sb[:, QT + c0 : QT + c1], pv2[64:65, QT + c0 : QT + c1]
                    )
                else:
                    nc.vector.tensor_copy(den_sb[:, c0:c1], pv2[64:65, c0:c1])
                    nc.vector.tensor_copy(
                        den_sb[:, QT + c0 : QT + c1], pv2[64:65, QT + c0 : QT + c1]
                    )
                den_v = den_sb[:].rearrange("p (b q) -> p b q", b=2)[:, :, c0:c1]
                seed0 = bcseed[0:1, :].rearrange("p (b q) -> p b q", b=2)[
                    :, :, c0:c1
                ]
                seed32 = bcseed[32:33, :].rearrange("p (b q) -> p b q", b=2)[
                    :, :, c0:c1
                ]
                nc.vector.reciprocal_approx_fast(seed0, den_v)
                nc.vector.tensor_copy(seed32, seed0)
                bcast = normp.tile([64, 2 * QT], f32, name="bcast", tag="bcast")
                bc_v = bcast[:].rearrange("p (b q) -> p b q", b=2)[:, :, c0:c1]
                sd_v = bcseed[:].rearrange("p (b q) -> p b q", b=2)[:, :, c0:c1]
                nc.vector.stream_shuffle(bc_v, sd_v, [0] * 32)
                nc.vector.tensor_mul(
                    aots[qj][0:64, c0:c1], pv2[0:64, c0:c1], bcast[:, c0:c1]
                )
                nc.vector.tensor_mul(
                    aots[qj][64:128, c0:c1], pv2[0:64, QT + c0 : QT + c1],
                    bcast[:, QT + c0 : QT + c1],
                )

            def dve_exp_blocks(qi, n_kb):
                cnt = DVE_EXP_CNT.get(qi, 0)
                return {
                    kb
                    for kb in range(n_kb)
                    if ((kb + 1) * cnt) // n_kb > (kb * cnt) // n_kb
                }

            # ---- prologue: K/Q projections for q-tile 0 (dense PE warmup);
            # V blocks 0-3 are fillers inside the tile-0 loop.
            for f in kq_units(kts[0], wk_sb, 0, cast_eng="v"):
                f()
            for f in kq_units(qts[0], wq_sb, 0, cast_eng="v"):
                f()

            # both heads' PV accumulators in ONE persistent psum tile
            # (2 banks): head A cols 0:512, head B cols 512:1024.  Row 65 is
            # a scratch row for wait-free PE "warmer" matmuls; persistence
            # (no pool rotation) is what keeps the warmers dependency-free.
            pv2 = pvp.tile([65, 2 * QT], f32, name="pv2", tag="pv")

            def pe_warm(n, cols=128):
                # wait-free dummy LDWEIGHTS: keep the PE's activity monitor
                # busy across known dependency stalls so the HAM clock gate
                # doesn't halve the clock (re-warming after a ~3.4us idle
                # window costs 10-30us at K=4/8).  Safe because every real
                # matmul loads its own weights right before it executes.
                for _ in range(n):
                    nc.tensor.ldweights(wk_sb[:, 0:cols])

            # ---- main loop over q-tiles ----
            # o_proj work is BANKED: each tile contributes 8 units when its
            # aot is ready, but late tiles consume more — they are the ones
            # whose exp volume leaves the PE short of filler work.
            oproj_backlog = deque()
            # tile 7 keeps only 4 as block fillers — the other 8 interleave
            # into the tail chunks to keep the PE continuously busy (HAM!)
            OPROJ_CONSUME = {1: 4, 2: 4, 3: 4, 4: 8, 5: 12, 6: 12, 7: 4}
            for qi in range(NQ):
                if qi <= 2:
                    x_wave(qi + 1)
                if qi >= 5:
                    pe_warm(8)
                fillers = deque()
                if qi == 0:
                    for kb in range(4):
                        fillers.extend(v_units(kb, cast_eng="v"))
                if qi + 1 < NQ:
                    fillers.extend(proj_units(qi + 1))
                if qi >= 1:
                    for sbi in range(4):
                        for half in range(2):
                            oproj_backlog.append((qi - 1, sbi, half))
                    n_cons = min(OPROJ_CONSUME[qi], len(oproj_backlog))
                    for ci in range(n_cons):
                        qj, sbi, half = oproj_backlog.popleft()
                        ce = "v" if qi <= 4 else ("s" if ci % 2 == 0 else "v")
                        de = nc.gpsimd if ci % 2 else nc.sync
                        fillers.append(oproj_unit(qj, sbi, half, ce, de))

                n_kb = 4 * (qi + 1)
                if qi == 0:
                    # bf16 path for the first q-tile (rows 0-511): every
                    # block is diagonal; per-block M=65 PV, no DoubleRow.
                    for kb in range(4):
                        kvs = slice(kb * KB, (kb + 1) * KB)
                        off = kb
                        qlo = off * KB
                        nq = QT - qlo
                        lg = lgp.tile([128, 2 * QT], f32, name="lg", tag="lg")
                        nc.tensor.matmul(
                            lg[:, qlo:QT], lhsT=kts[0][0:64, kvs],
                            rhs=qts[0][0:64, qlo:QT], start=True, stop=True,
                        )
                        nc.tensor.matmul(
                            lg[:, QT + qlo : 2 * QT], lhsT=kts[0][64:128, kvs],
                            rhs=qts[0][64:128, qlo:QT], start=True, stop=True,
                        )
                        exb = expp.tile([128, 2 * QT], bf16, name="exb", tag="exp")
                        lg_v = lg[:].rearrange("p (h q) -> p h q", h=2)[:, :, qlo:QT]
                        exb_v = exb[:].rearrange("p (h q) -> p h q", h=2)[:, :, qlo:QT]
                        nc.scalar.activation(exb_v, lg_v, Exp, scale=SCALE,
                                             bias=ebias[:])
                        if fillers:
                            n_pop = math.ceil(len(fillers) / (4 - kb))
                            for _ in range(n_pop):
                                fillers.popleft()()
                        # only the 128 columns crossing the diagonal need
                        # masking; later columns are fully kept as-is
                        exb_w = exb[:].rearrange("p (h q) -> p h q", h=2)[
                            :, :, qlo : qlo + KB
                        ]
                        nc.gpsimd.affine_select(
                            out=exb_w, in_=exb_w,
                            compare_op=mybir.AluOpType.is_ge,
                            fill=0.0, base=0,
                            pattern=[[0, 2], [1, KB]],
                            channel_multiplier=-1,
                        )
                        nc.tensor.matmul(
                            pv2[0:65, qlo:QT], lhsT=vb16[kb][:, 0:65],
                            rhs=exb[:, qlo:QT],
                            start=(kb == 0), stop=(kb == 3),
                            skip_group_check=True,
                        )
                        nc.tensor.matmul(
                            pv2[0:65, QT + qlo : 2 * QT], lhsT=vb16[kb][:, 65:130],
                            rhs=exb[:, QT + qlo : 2 * QT],
                            start=(kb == 0), stop=(kb == 3),
                            skip_group_check=True,
                        )
                    n_kb = 0  # skip the fp8 loop below

                dve_set = dve_exp_blocks(qi, n_kb)
                ex8 = None
                pending_pvs = deque()
                for kb in range(n_kb):
                    # logits for both heads of kv-block kb: head A -> cols
                    # 0:512 (PSUM bank 0), head B -> cols 512:1024 (bank 1).
                    # Row-group packing (rows 0-63 / 64-127) runs the two
                    # matmuls concurrently in the PE array.
                    # Diagonal-band pairs: columns q < qlo_p = (pair off)*128
                    # are entirely masked for both members -> skip them.
                    kvs = slice((kb % 4) * KB, (kb % 4 + 1) * KB)
                    ktile = kts[kb // 4]
                    r = kb % 2
                    off = kb - 4 * qi
                    qlo = max(off - r, 0) * KB  # pair-aligned trim
                    nq = QT - qlo
                    lg = lgp.tile([128, 2 * QT], f32, name="lg", tag="lg")
                    nc.tensor.matmul(
                        lg[:, qlo:QT], lhsT=ktile[0:64, kvs],
                        rhs=qts[qi][0:64, qlo:QT],
                        start=True, stop=True,
                    )
                    nc.tensor.matmul(
                        lg[:, QT + qlo : 2 * QT], lhsT=ktile[64:128, kvs],
                        rhs=qts[qi][64:128, qlo:QT],
                        start=True, stop=True,
                    )
                    if r == 0:
                        # exp tile for this kv pair: [128, (head, slot, q)]
                        ex8 = expp.tile([128, 4 * QT], fp8, name="ex8", tag="exp")
                    exv = ex8[:].rearrange("p (h t q) -> p h t q", h=2, t=2)
                    lg_v = lg[:].rearrange("p (h q) -> p h q", h=2)[:, :, qlo:QT]
                    ex_v = exv[:, :, r, qlo:QT]
                    if kb in dve_set:
                        # Schraudolph on DVE: int8 code IS the e5m2 encoding
                        nc.vector.tensor_scalar(
                            out=ex_v.bitcast(i8),
                            in0=lg_v,
                            scalar1=SCH_A,
                            scalar2=SCH_B,
                            op0=Mult,
                            op1=Add,
                        )
                    else:
                        nc.scalar.activation(ex_v, lg_v, Exp, scale=SCALE,
                                             bias=ebias[:])

                    # emit PV pairs TWO pairs behind their exp: the PE
                    # reaches them well after both engines' exps finished,
                    # so the in-order PE queue never stalls on exp (which
                    # would delay logits fills and bubble ACT/DVE)
                    if r == 1 and len(pending_pvs) >= 2:
                        pending_pvs.popleft()()

                    # PE filler work while ACT/DVE compute exp
                    if fillers:
                        n_pop = math.ceil(len(fillers) / (n_kb - kb))
                        for _ in range(n_pop):
                            fillers.popleft()()

                    if off >= 0:
                        # causal mask on GPSIMD: keep where q - kv - off*128
                        # >= 0 else 0; with q = qlo + j this is
                        # j + qlo - off*128 - kv >= 0.  Only columns up to
                        # the end of the diagonal band (q < off*128+128) can
                        # be masked — don't touch the fully-kept tail.
                        w = min(off * KB + KB - qlo, nq)
                        ex_w = exv[:, :, r, qlo : qlo + w]
                        nc.gpsimd.affine_select(
                            out=ex_w,
                            in_=ex_w,
                            compare_op=mybir.AluOpType.is_ge,
                            fill=0.0,
                            base=qlo - off * KB,
                            pattern=[[0, 2], [1, w]],
                            channel_multiplier=-1,
                        )
                    if r == 1:
                        # DoubleRow PV over the kv pair (contraction 256):
                        # lhsT [128, 2, 65], rhs [128, 2, nq] -> out [65, nq]
                        def make_pv(kp=kb // 2, qlo=qlo, exv=exv, last=(kb == n_kb - 1)):
                            def f():
                                vv = v8s[kp][:].rearrange("p (t d) -> p t d", t=2)
                                nc.tensor.matmul(
                                    pv2[0:65, qlo:QT], lhsT=vv[:, :, 0:65],
                                    rhs=exv[:, 0, :, qlo:QT],
                                    start=(kp == 0), stop=last,
                                    perf_mode=DoubleRow,
                                    skip_group_check=True,
                                )
                                nc.tensor.matmul(
                                    pv2[0:65, QT + qlo : 2 * QT], lhsT=vv[:, :, 80:145],
                                    rhs=exv[:, 1, :, qlo:QT],
                                    start=(kp == 0), stop=last,
                                    perf_mode=DoubleRow,
                                    skip_group_check=True,
                                )

                            return f

                        pending_pvs.append(make_pv())
                if qi >= 4:
                    pe_warm(5)
                if qi == NQ - 1 and len(pending_pvs) == 2:
                    # flush only the second-to-last pair: it finalizes pv
                    # columns 0:256 (the last diagonal pair writes 256:512
                    # only), so the first half of the tail can overlap it
                    pending_pvs.popleft()()
                else:
                    while pending_pvs:
                        pending_pvs.popleft()()
                while fillers:
                    fillers.popleft()()
                # normalize: aot = pv[0:64] / pv[64]; the first half's o_proj
                # for the final tile starts while the second half normalizes
                if qi == NQ - 1:
                    # pipeline the tail in 128-col chunks: each chunk's
                    # normalize feeds its own o_proj unit immediately, and
                    # the final DMAs fan out over three queues.
                    tail_dma = [nc.gpsimd, nc.sync, nc.scalar]
                    for sbi in range(4):
                        if sbi == 2:
                            # columns 256:512 need the last diagonal pair —
                            # its exp/PV overlapped the first two chunks
                            while pending_pvs:
                                pending_pvs.popleft()()
                        norm_emit(pv2, qi, sbi * 128, (sbi + 1) * 128)
                        for half in range(2):
                            de = tail_dma[(2 * sbi + half) % 3]
                            oproj_unit(qi, sbi, half, "s", de)()
                        pe_warm(3)
                        # interleave banked o_proj work between tail chunks
                        # so the PE never idles long enough to re-throttle
                        for _ in range(2):
                            if oproj_backlog:
                                qj, sb_, hf = oproj_backlog.popleft()
                                oproj_unit(qj, sb_, hf, "s", tail_dma[sbi % 3])()
                else:
                    norm_emit(pv2, qi, 0, QT)

    nc.compile()
    return nc


def _host_inputs(x, Wq, Wk, Wv, Wo):
    x2 = np.asarray(x, dtype=np.float32).reshape(S, E)
    xT = np.ascontiguousarray(x2.T).astype(BF16)

    in_maps = []
    for c in range(NCORES):
        r = slice(128 * c, 128 * (c + 1))

        def pack(wT):  # [1024(e), 128(d)] -> [128(p), ec*128+d]
            return np.ascontiguousarray(
                wT.reshape(8, 128, 128).transpose(1, 0, 2).reshape(128, 1024)
            ).astype(BF16)

        wq_c = pack(np.asarray(Wq, np.float32)[r, :].T)
        wk_c = pack(np.asarray(Wk, np.float32)[r, :].T)
        wv_c = pack(np.asarray(Wv, np.float32)[r, :].T)
        wo_c = np.ascontiguousarray(np.asarray(Wo, np.float32)[:, r].T).astype(BF16)
        in_maps.append(
            {
                "xT": xT,
                "wq": wq_c,
                "wk": wk_c,
                "wv": wv_c,
                "wo": wo_c,
            }
        )
    return in_maps


def _get_nc():
    if "nc" not in _CACHE:
        _CACHE["nc"] = _build_nc()
    return _CACHE["nc"]


def run(x, Wq, Wk, Wv, Wo, trace=False, trace_kwargs=None):
    """Build+run the SPMD kernel; returns (full_output [S,E] f32, BassKernelResults)."""
    from concourse.bass_utils import run_bass_kernel_spmd

    nc = _get_nc()
    in_maps = _host_inputs(x, Wq, Wk, Wv, Wo)
    res = run_bass_kernel_spmd(
        nc,
        in_maps,
        list(range(NCORES)),
        trace=trace,
        **(trace_kwargs or {}),
    )
    out = np.zeros((S, E), dtype=np.float32)
    for c in range(NCORES):
        out += res.results[c]["out"].astype(np.float32)
    return out, res


def kernel(x, Wq, Wk, Wv, Wo):
    out, _ = run(x, Wq, Wk, Wv, Wo)
    return out.reshape(1, S, E).astype(np.float32)
